# revision 1
# baseline (speedup 1.0000x reference)
"""Discrete Hawkes conditional-intensity kernel for 8 Trainium2 NeuronCores.

Math
----
Reference computes, per query i with (t, s) = (t_i, s_i):

    lam_i = clip(mu[s] + alpha[s, s] * b * F[t, s], 1e-5)
    F[t, s] = sum_{tp < t} obs[tp, s] * exp(-b * (t - tp))

F obeys F[t+1] = e * (F[t] + obs[t]), e = exp(-b), i.e. it is an
exponentially-decayed prefix sum over time.  On device we build the full
table G[t, s] = mu[s] + alpha[s,s]*b*F[t, s] with a blocked formulation
(time blocks of 128 on the PE array + a 32-step cross-block carry), store
it to DRAM, then answer the 8192 queries per core with one indirect-DMA
element gather G_flat[t*256 + s].

Sharding: queries (t, s) are split 8x8192 across cores (data parallel);
obs / mu / alpha / beta are replicated.  No collectives needed.
"""

import os
import sys

import numpy as np

_REPO_CANDIDATES = ("/opt/trn_rl_repo", os.path.expanduser("~/.axon_site/_ro/trn_rl_repo"))
for _p in _REPO_CANDIDATES:
    if os.path.isdir(_p) and _p not in sys.path:
        sys.path.append(_p)

import concourse.bass as bass
import concourse.tile as tile
from concourse import bacc, mybir
from concourse.bass_utils import run_bass_kernel_spmd

# Problem constants (hardcoded per spec).
N_TIME = 4096
N_SPACE = 256
BATCH = 65536
N_CORES = 8
LAM_MIN = 1e-5

P = 128               # partitions / time-block size
J = N_TIME // P       # 32 time blocks
PER_CORE = BATCH // N_CORES   # 8192 queries per core
CH = 512              # matmul N-chunk (one PSUM bank)
NCH = (J * N_SPACE) // CH     # 16 chunks over the (j, s) flat axis
# gather slot layout: columns are staged by the largest t they may contain,
# so early columns can gather as soon as the matching part of G is stored.
# quarter k (cap COLS[k] columns) only holds queries with t < QBOUND[k].
COLS = (6, 7, 13, 13, 25)
QBOUND = (512, 1024, 2048, 3072, 4096)
FQ = sum(COLS)                # 64 query slots per partition
NSLOT = P * FQ                # 8192 slots per core

f32 = mybir.dt.float32
bf16 = mybir.dt.bfloat16
i32 = mybir.dt.int32
Alu = mybir.AluOpType
Act = mybir.ActivationFunctionType


def build_nc():
    nc = bacc.Bacc("TRN2", target_bir_lowering=False, debug=False)

    t_h = nc.dram_tensor("t", [NSLOT], i32, kind="ExternalInput")
    s_h = nc.dram_tensor("s", [NSLOT], i32, kind="ExternalInput")
    obs_h = nc.dram_tensor("obs", [N_TIME, N_SPACE], i32, kind="ExternalInput")
    mu_h = nc.dram_tensor("mu", [N_SPACE], f32, kind="ExternalInput")
    alpha_h = nc.dram_tensor("alpha", [N_SPACE, N_SPACE], f32, kind="ExternalInput")
    beta_h = nc.dram_tensor("beta", [1], f32, kind="ExternalInput")
    g_h = nc.dram_tensor("gtab", [N_TIME * N_SPACE + 2], f32, kind="Internal")
    out_h = nc.dram_tensor("out", [NSLOT], f32, kind="ExternalOutput")

    from contextlib import ExitStack

    with tile.TileContext(nc) as tc, ExitStack() as ctx:
        sb = ctx.enter_context(tc.tile_pool(name="sb", bufs=1))
        ps = ctx.enter_context(tc.tile_pool(name="ps", bufs=4, space="PSUM"))
        psr = ctx.enter_context(tc.tile_pool(name="psr", bufs=2, space="PSUM"))
        ps1 = ctx.enter_context(tc.tile_pool(name="ps1", bufs=1, space="PSUM"))
        sb2 = ctx.enter_context(tc.tile_pool(name="sb2", bufs=4))

        # ---- input loads -------------------------------------------------
        obs_view = obs_h.ap().rearrange("(j p) s -> p j s", p=P)
        obs_i = sb.tile([P, J, N_SPACE], i32)
        for q in range(8):
            nc.sync.dma_start(obs_i[:, 4 * q:4 * q + 4, :],
                              obs_view[:, 4 * q:4 * q + 4, :])

        beta_bc = sb.tile([P, 1], f32)
        nc.scalar.dma_start(beta_bc[:], bass.AP(beta_h, 0, [[0, P], [1, 1]]))

        adiag = sb.tile([1, N_SPACE], f32)
        nc.scalar.dma_start(adiag[:], bass.AP(alpha_h, 0, [[0, 1], [N_SPACE + 1, N_SPACE]]))

        rhs2 = sb.tile([2, J * N_SPACE], bf16)  # row0 = carry C flat, row1 = mu tiled
        mu_f = sb.tile([1, N_SPACE], f32)
        nc.scalar.dma_start(mu_f[:], bass.AP(mu_h, 0, [[0, 1], [1, N_SPACE]]))
        mu_b = sb.tile([1, N_SPACE], bf16)
        nc.vector.tensor_copy(mu_b[:], mu_f[:])
        nc.scalar.dma_start(
            rhs2[1:2, :].rearrange("o (j s) -> o j s", s=N_SPACE),
            mu_b[:].unsqueeze(1).broadcast_to((1, J, N_SPACE)))

        tq = sb.tile([P, FQ], i32)
        nc.scalar.dma_start(tq[:], bass.AP(t_h, 0, [[FQ, P], [1, FQ]]))
        sq = sb.tile([P, FQ], i32)
        nc.scalar.dma_start(sq[:], bass.AP(s_h, 0, [[FQ, P], [1, FQ]]))

        # ---- runtime constants from beta --------------------------------
        negb = sb.tile([P, 1], f32)
        nc.vector.tensor_scalar(out=negb[:], in0=beta_bc[:], scalar1=-1.0,
                                scalar2=None, op0=Alu.mult)
        negb128 = sb.tile([P, 1], f32)
        nc.vector.tensor_scalar(out=negb128[:], in0=negb[:], scalar1=128.0,
                                scalar2=None, op0=Alu.mult)

        # LdT[tp, m] = exp(-b (m - tp)) for tp < m else 0   (within-block decay)
        xd = sb.tile([P, P], i32)
        nc.gpsimd.iota(xd[:], [[1, P]], base=0, channel_multiplier=-1)   # f - p
        lda = sb.tile([P, P], f32)
        nc.vector.tensor_scalar(out=lda[:], in0=xd[:], scalar1=negb[:],
                                scalar2=None, op0=Alu.mult)
        ldb = sb.tile([P, P], f32)
        nc.vector.tensor_scalar(out=ldb[:], in0=xd[:], scalar1=1000.0,
                                scalar2=-1000.0, op0=Alu.mult, op1=Alu.add)
        ldm = sb.tile([P, P], f32)
        nc.vector.tensor_tensor(out=ldm[:], in0=lda[:], in1=ldb[:], op=Alu.min)
        ldt = sb.tile([P, P], f32)
        nc.scalar.activation(ldt[:], ldm[:], Act.Exp)
        ldtb = sb.tile([P, P], bf16)
        nc.vector.tensor_copy(ldtb[:], ldt[:])

        # v[tp] = exp(-b (128 - tp))  (end-of-block carry weights)
        xv = sb.tile([P, 1], i32)
        nc.gpsimd.iota(xv[:], [[0, 1]], base=P, channel_multiplier=-1)   # 128 - p
        vm = sb.tile([P, 1], f32)
        nc.vector.tensor_scalar(out=vm[:], in0=xv[:], scalar1=negb[:],
                                scalar2=None, op0=Alu.mult)
        vv = sb.tile([P, 1], f32)
        nc.scalar.activation(vv[:], vm[:], Act.Exp)
        vvb = sb.tile([P, 1], bf16)
        nc.vector.tensor_copy(vvb[:], vv[:])

        # LcT[k, j] = exp(-128 b (j - 1 - k)) for k <= j-1 else 0  (carry matrix)
        xc = sb.tile([J, J], i32)
        nc.gpsimd.iota(xc[:], [[1, J]], base=-1, channel_multiplier=-1)  # f - 1 - p
        lca = sb.tile([J, J], f32)
        nc.vector.tensor_scalar(out=lca[:], in0=xc[:], scalar1=negb128[:J, :],
                                scalar2=None, op0=Alu.mult)
        lcb = sb.tile([J, J], f32)
        nc.vector.tensor_scalar(out=lcb[:], in0=xc[:], scalar1=1000.0,
                                scalar2=None, op0=Alu.mult)
        lcm = sb.tile([J, J], f32)
        nc.vector.tensor_tensor(out=lcm[:], in0=lca[:], in1=lcb[:], op=Alu.min)
        lct = sb.tile([J, J], f32)
        nc.scalar.activation(lct[:], lcm[:], Act.Exp)

        # u2: row0 = u_i = exp(-b i), row1 = ones (mu term).
        # scale vector [-b; 0] makes exp produce both rows at once.
        negb01 = sb.tile([2, 1], f32)
        nc.vector.memset(negb01[:], 0.0)
        nc.vector.tensor_copy(negb01[0:1, :], negb[0:1, :])
        xu = sb.tile([2, P], i32)
        nc.gpsimd.iota(xu[:], [[1, P]], base=0, channel_multiplier=0)    # f
        um = sb.tile([2, P], f32)
        nc.vector.tensor_scalar(out=um[:], in0=xu[:], scalar1=negb01[:],
                                scalar2=None, op0=Alu.mult)
        u2 = sb.tile([2, P], f32)
        nc.scalar.activation(u2[:], um[:], Act.Exp)
        u2b = sb.tile([2, P], bf16)
        nc.vector.tensor_copy(u2b[:], u2[:])

        # asb[s] = b * alpha[s, s], broadcast to all 128 partitions via PE
        asb_row = sb.tile([1, N_SPACE], f32)
        nc.vector.tensor_scalar(out=asb_row[:], in0=adiag[:],
                                scalar1=beta_bc[:1, :], scalar2=None, op0=Alu.mult)
        ones1 = sb.tile([1, P], f32)
        nc.vector.memset(ones1[:], 1.0)
        asb_ps = ps1.tile([P, N_SPACE], f32)
        nc.tensor.matmul(asb_ps[:], lhsT=ones1[:], rhs=asb_row[:], start=True, stop=True)
        asb_bc = sb.tile([P, N_SPACE], f32)
        nc.vector.tensor_copy(asb_bc[:], asb_ps[:])

        # obs_f[tp, j, s] = obs * asb[s]   (convert + scale, 4 chunked DVE passes)
        obs_f = sb.tile([P, J * N_SPACE], bf16)
        obs_ff = obs_f[:]                # [P, 8192] flat view
        obs_f3 = obs_f[:].rearrange("p (j s) -> p j s", s=N_SPACE)
        for q in range(4):
            nc.vector.tensor_tensor(
                out=obs_f3[:, 8 * q:8 * q + 8, :],
                in0=obs_i[:, 8 * q:8 * q + 8, :],
                in1=asb_bc[:].unsqueeze(1).broadcast_to((P, 8, N_SPACE)),
                op=Alu.mult,
            )

        # ---- fused quarter pipeline ------------------------------------
        # For each t-quarter k: reduce r over its 4 obs chunks, extend the
        # carry, build + store its 4 G chunks, then immediately issue the
        # gather columns that only touch t < QBOUND[k].  This keeps the
        # Pool queue (the serial bottleneck) fed as early as possible.
        r_flat = sb.tile([1, J * N_SPACE], f32)
        r32 = sb.tile([J, N_SPACE], f32)
        rhs2_j = rhs2[0:1, :].rearrange("o (j s) -> o j s", s=N_SPACE)
        g_store = bass.AP(g_h, 0, [[N_SPACE, P], [P * N_SPACE, J], [1, N_SPACE]])

        idx1 = sb.tile([P, FQ], i32)
        nc.vector.tensor_scalar(out=idx1[:], in0=tq[:], scalar1=8,
                                scalar2=None, op0=Alu.arith_shift_left)
        idx = sb.tile([P, FQ], i32)
        nc.vector.tensor_tensor(out=idx[:], in0=idx1[:], in1=sq[:], op=Alu.add)

        gath = sb.tile([P, 2 * FQ], f32)
        views = [bass.AP(g_h, 0, [[1, QBOUND[k] * N_SPACE], [1, 1]])
                 for k in range(4)]
        views.append(bass.AP(g_h, 0, [[1, N_TIME * N_SPACE + 2], [1, 1]]))
        zpad = sb.tile([1, 2], f32)
        nc.vector.memset(zpad[:], 0.0)
        nc.sync.dma_start(bass.AP(g_h, N_TIME * N_SPACE, [[1, 1], [1, 2]]), zpad[:])

        fbase = 0
        for k in range(4):
            for c in range(4 * k, 4 * k + 4):
                r_ps = psr.tile([1, CH], f32)
                nc.tensor.matmul(r_ps[:], lhsT=vvb[:],
                                 rhs=obs_ff[:, c * CH:(c + 1) * CH],
                                 start=True, stop=True)
                nc.scalar.activation(r_flat[:, c * CH:(c + 1) * CH], r_ps[:],
                                     Act.Copy)
            nc.sync.dma_start(r32[8 * k:8 * k + 8, :],
                              r_flat[:, 2048 * k:2048 * (k + 1)])
            c_ps = ps1.tile([8, N_SPACE], f32, tag="cps")
            nc.tensor.matmul(c_ps[:], lhsT=lct[0:8 * (k + 1), 8 * k:8 * (k + 1)],
                             rhs=r32[0:8 * (k + 1), :], start=True, stop=True)
            c32 = sb2.tile([8, N_SPACE], bf16, tag="c32")
            nc.vector.tensor_copy(c32[:], c_ps[:])
            nc.sync.dma_start(rhs2_j[:, 8 * k:8 * k + 8, :], c32[:])

            for c in range(4 * k, 4 * k + 4):
                pch = ps.tile([P, CH], f32)
                nc.tensor.matmul(pch[:], lhsT=ldtb[:],
                                 rhs=obs_ff[:, c * CH:(c + 1) * CH],
                                 start=True, stop=True)
                nc.tensor.matmul(pch[:], lhsT=u2b[:],
                                 rhs=rhs2[:, c * CH:(c + 1) * CH],
                                 start=False, stop=True, skip_group_check=True)
                gch = sb2.tile([P, CH], f32, tag="gch")
                if c % 2 == 0:
                    nc.vector.tensor_copy(gch[:], pch[:])
                else:
                    nc.scalar.activation(gch[:], pch[:], Act.Copy)
                jj = c * CH // N_SPACE
                eng = nc.sync if c % 2 == 0 else nc.scalar
                eng.dma_start(g_store[:, jj:jj + CH // N_SPACE, :], gch[:])

            stages = [0, 1] if k == 0 else [k + 1]
            for st in stages:
                for f in range(fbase, fbase + COLS[st]):
                    nc.gpsimd.indirect_dma_start(
                        out=gath[:, 2 * f:2 * f + 2],
                        out_offset=None,
                        in_=views[st],
                        in_offset=bass.IndirectOffsetOnAxis(ap=idx[:, f:f + 1],
                                                            axis=0),
                    )
                cols = COLS[st]
                lam = sb2.tile([P, FQ], f32, tag="lam")
                nc.vector.tensor_scalar(
                    out=lam[:, :cols].rearrange("p (f o) -> p f o", o=1),
                    in0=gath[:].rearrange("p (f o) -> p f o", o=2)[
                        :, fbase:fbase + cols, 0:1],
                    scalar1=float(LAM_MIN), scalar2=None, op0=Alu.max)
                nc.scalar.dma_start(
                    bass.AP(out_h, fbase, [[FQ, P], [1, cols]]), lam[:, :cols])
                fbase += cols

    nc.compile()
    return nc


_NC_CACHE = None


def _get_nc():
    global _NC_CACHE
    if _NC_CACHE is None:
        _NC_CACHE = build_nc()
    return _NC_CACHE


def kernel(t, s, obs, mu, alpha, beta, **_unused):
    t = np.ascontiguousarray(np.asarray(t, dtype=np.int32))
    s = np.ascontiguousarray(np.asarray(s, dtype=np.int32))
    obs = np.ascontiguousarray(np.asarray(obs, dtype=np.int32))
    mu = np.ascontiguousarray(np.asarray(mu, dtype=np.float32))
    alpha = np.ascontiguousarray(np.asarray(alpha, dtype=np.float32))
    beta = np.ascontiguousarray(np.asarray(beta, dtype=np.float32))

    nc = _get_nc()
    in_maps, perms = [], []
    for c in range(N_CORES):
        sl = slice(c * PER_CORE, (c + 1) * PER_CORE)
        tc_, sc_ = t[sl], s[sl]
        t_dev, s_dev, perm = _route_queries(tc_, sc_)
        perms.append(perm)
        in_maps.append({
            "t": t_dev, "s": s_dev,
            "obs": obs, "mu": mu, "alpha": alpha, "beta": beta,
        })
    res = run_bass_kernel_spmd(nc, in_maps, core_ids=list(range(N_CORES)))
    outs = []
    for c in range(N_CORES):
        dev = res.results[c]["out"]          # [NSLOT]
        o = np.empty(PER_CORE, np.float32)
        o[perms[c][1]] = dev[perms[c][0]]
        outs.append(o)
    return np.concatenate(outs).astype(np.float32)


def _route_queries(tc_, sc_):
    """Assign the core's queries to gather slots.

    Slot (p, f) holds device position p*FQ + f; gather column f covers the
    128 slots with that f.  Columns < COLS_A must only hold t < 2048
    queries (their gathers race the second table half).  Unused slots get a
    harmless (t=0, s=0) dummy.  Returns (dev_pos, orig_pos) so that
    out[orig_pos] = dev_out[dev_pos].
    """
    n = tc_.shape[0]
    order = np.argsort(tc_, kind="stable")      # queries by ascending t
    ts = tc_[order]
    t_dev = np.zeros(NSLOT, np.int32)
    s_dev = np.zeros(NSLOT, np.int32)
    dev_parts, orig_parts = [], []
    lo = 0
    fbase = 0
    nst = len(COLS)
    for k in range(nst):
        cap = P * COLS[k]
        # queries eligible for stage k that are not yet placed
        hi = np.searchsorted(ts, QBOUND[k], side="left")
        take = min(cap, hi - lo) if k < nst - 1 else (n - lo)
        if k == nst - 1 and take > cap:
            raise RuntimeError("query t-distribution infeasible for slot layout")
        sel = order[lo:lo + take]
        kk = np.arange(take)
        dev = (kk % P) * FQ + (fbase + kk // P)
        dev_parts.append(dev)
        orig_parts.append(sel)
        lo += take
        fbase += COLS[k]
    dev_pos = np.concatenate(dev_parts)
    orig_pos = np.concatenate(orig_parts)
    t_dev[dev_pos] = tc_[orig_pos]
    s_dev[dev_pos] = sc_[orig_pos]
    return t_dev, s_dev, (dev_pos, orig_pos)


if __name__ == "__main__":
    # quick self-check against a numpy re-implementation on random data
    rng = np.random.default_rng(0)
    t = rng.integers(0, N_TIME, BATCH).astype(np.int32)
    s = rng.integers(0, N_SPACE, BATCH).astype(np.int32)
    obs = rng.integers(0, 10, (N_TIME, N_SPACE)).astype(np.int32)
    mu = rng.random(N_SPACE, dtype=np.float32)
    alpha = rng.random((N_SPACE, N_SPACE), dtype=np.float32)
    beta = (rng.random(1, dtype=np.float32) + 0.1).astype(np.float32)

    got = kernel(t=t, s=s, obs=obs, mu=mu, alpha=alpha, beta=beta)

    b = float(beta[0])
    e = np.exp(-b)
    F = np.zeros((N_TIME, N_SPACE), np.float64)
    for tt in range(1, N_TIME):
        F[tt] = e * (F[tt - 1] + obs[tt - 1])
    G = np.clip(mu[None, :] + np.diag(alpha)[None, :] * b * F, LAM_MIN, None)
    want = G[t, s].astype(np.float32)
    err = np.abs(got - want) / np.maximum(np.abs(want), 1e-6)
    print("max rel err:", err.max(), "mean:", err.mean())



# revision 6
# speedup vs baseline: 1.8767x; 1.8767x over previous
"""Discrete Hawkes conditional-intensity kernel for 8 Trainium2 NeuronCores.

Math
----
Reference computes, per query i with (t, s) = (t_i, s_i):

    lam_i = clip(mu[s] + alpha[s, s] * b * F[t, s], 1e-5)
    F[t, s] = sum_{tp < t} obs[tp, s] * exp(-b * (t - tp))

F obeys F[t+1] = e * (F[t] + obs[t]), e = exp(-b): an exponentially-decayed
prefix sum over time.  On device each core builds the table
G[t, sl] = mu[sl] + alpha[sl,sl]*b*F[t, sl] for its 32 s-columns with a
blocked formulation (time blocks of 128 on the PE array + a 32-step
cross-block carry), keeping G in SBUF as [128 (t&127), 1024 ((t>>7)*32+sl)].

Sharding: by SPACE.  Core c owns s-columns [32c, 32c+32) (the time scan is
core-local, so no collectives).  Queries are routed host-side to the core
owning their s value.

Gather: gpsimd ap_gather reads G directly from SBUF -- one instruction for
all ~8.2k queries of a core (no DRAM round-trip, no per-element DMA
descriptors).  ap_gather shares one index list per 16-partition group, so a
query's value appears on all 16 partitions of its group; a host-supplied
one-hot mask + an 8-column matmul compress the group dimension down to the
single wanted value per query.
"""

import os
import sys

import numpy as np

_REPO_CANDIDATES = ("/opt/trn_rl_repo", os.path.expanduser("~/.axon_site/_ro/trn_rl_repo"))
for _p in _REPO_CANDIDATES:
    if os.path.isdir(_p) and _p not in sys.path:
        sys.path.append(_p)

import ml_dtypes
import concourse.bass as bass
import concourse.tile as tile
from concourse import bacc, mybir
from concourse.bass_utils import run_bass_kernel_spmd

# Problem constants (hardcoded per spec).
N_TIME = 4096
N_SPACE = 256
BATCH = 65536
N_CORES = 8
LAM_MIN = 1e-5

P = 128                  # partitions / time-block size
J = N_TIME // P          # 32 time blocks
SL = N_SPACE // N_CORES  # 32 s-columns per core
XW = J * SL              # 1024 free elements of the G table per partition
NG = 8                   # gpsimd 16-partition groups
NI = 1392                # gather slots per group (mean load 1024, wide margin)
NSLOT = NG * NI          # device output length

f32 = mybir.dt.float32
bf16 = mybir.dt.bfloat16
i32 = mybir.dt.int32
i16 = mybir.dt.int16
i8 = mybir.dt.int8
Alu = mybir.AluOpType
Act = mybir.ActivationFunctionType


def build_nc():
    nc = bacc.Bacc("TRN2", target_bir_lowering=False, debug=False)

    obs_h = nc.dram_tensor("obsr", [P, XW], i8, kind="ExternalInput")
    idx_h = nc.dram_tensor("idxs", [P, NI // 16], i16, kind="ExternalInput")
    msk_h = nc.dram_tensor("mask", [P, NI], bf16, kind="ExternalInput")
    grp_h = nc.dram_tensor("grp", [P, NG], bf16, kind="ExternalInput")
    mus_h = nc.dram_tensor("mus", [SL], f32, kind="ExternalInput")
    ads_h = nc.dram_tensor("ads", [SL], f32, kind="ExternalInput")
    beta_h = nc.dram_tensor("beta", [1], f32, kind="ExternalInput")
    out_h = nc.dram_tensor("out", [NSLOT], f32, kind="ExternalOutput")

    from contextlib import ExitStack

    with tile.TileContext(nc) as tc, ExitStack() as ctx:
        sb = ctx.enter_context(tc.tile_pool(name="sb", bufs=1))
        ps = ctx.enter_context(tc.tile_pool(name="ps", bufs=2, space="PSUM"))
        psr = ctx.enter_context(tc.tile_pool(name="psr", bufs=2, space="PSUM"))
        ps1 = ctx.enter_context(tc.tile_pool(name="ps1", bufs=1, space="PSUM"))
        psc = ctx.enter_context(tc.tile_pool(name="psc", bufs=2, space="PSUM"))
        sb2 = ctx.enter_context(tc.tile_pool(name="sb2", bufs=4))

        # ---- input loads -------------------------------------------------
        obs_i = sb.tile([P, XW], i8)   # [p, (j, sl)] = obs[j*128+p, 32c+sl]
        nc.sync.dma_start(obs_i[:], obs_h.ap())

        idxt = sb.tile([P, NI // 16], i16)
        nc.sync.dma_start(idxt[:], idx_h.ap())
        mskt = sb.tile([P, NI], bf16)
        nc.scalar.dma_start(mskt[:], msk_h.ap())
        grpt = sb.tile([P, NG], bf16)
        nc.scalar.dma_start(grpt[:], grp_h.ap())

        beta_bc = sb.tile([P, 1], f32)
        nc.scalar.dma_start(beta_bc[:], bass.AP(beta_h, 0, [[0, P], [1, 1]]))
        mu_f = sb.tile([1, SL], f32)
        nc.scalar.dma_start(mu_f[:], bass.AP(mus_h, 0, [[0, 1], [1, SL]]))
        ads_f = sb.tile([1, SL], f32)
        nc.scalar.dma_start(ads_f[:], bass.AP(ads_h, 0, [[0, 1], [1, SL]]))

        # ---- runtime constants from beta --------------------------------
        negb = sb.tile([P, 1], f32)
        nc.vector.tensor_scalar(out=negb[:], in0=beta_bc[:], scalar1=-1.0,
                                scalar2=None, op0=Alu.mult)
        negb128 = sb.tile([P, 1], f32)
        nc.vector.tensor_scalar(out=negb128[:], in0=negb[:], scalar1=128.0,
                                scalar2=None, op0=Alu.mult)

        # LdT[tp, m] = exp(-b (m - tp)) for tp < m else 0   (within-block decay)
        xd = sb.tile([P, P], i32)
        nc.gpsimd.iota(xd[:], [[1, P]], base=0, channel_multiplier=-1)   # f - p
        lda = sb.tile([P, P], f32)
        nc.vector.tensor_scalar(out=lda[:], in0=xd[:], scalar1=negb[:],
                                scalar2=None, op0=Alu.mult)
        ldb = sb.tile([P, P], f32)
        nc.vector.tensor_scalar(out=ldb[:], in0=xd[:], scalar1=1000.0,
                                scalar2=-1000.0, op0=Alu.mult, op1=Alu.add)
        ldm = sb.tile([P, P], f32)
        nc.vector.tensor_tensor(out=ldm[:], in0=lda[:], in1=ldb[:], op=Alu.min)
        ldt = sb.tile([P, P], f32)
        nc.scalar.activation(ldt[:], ldm[:], Act.Exp)
        ldtb = sb.tile([P, P], bf16)
        nc.vector.tensor_copy(ldtb[:], ldt[:])

        # v[tp] = exp(-b (128 - tp))  (end-of-block carry weights)
        xv = sb.tile([P, 1], i32)
        nc.gpsimd.iota(xv[:], [[0, 1]], base=P, channel_multiplier=-1)   # 128 - p
        vm = sb.tile([P, 1], f32)
        nc.vector.tensor_scalar(out=vm[:], in0=xv[:], scalar1=negb[:],
                                scalar2=None, op0=Alu.mult)
        vv = sb.tile([P, 1], f32)
        nc.scalar.activation(vv[:], vm[:], Act.Exp)
        vvb = sb.tile([P, 1], bf16)
        nc.vector.tensor_copy(vvb[:], vv[:])

        # LcT[k, j] = exp(-128 b (j - 1 - k)) for k <= j-1 else 0  (carry matrix)
        xc = sb.tile([J, J], i32)
        nc.gpsimd.iota(xc[:], [[1, J]], base=-1, channel_multiplier=-1)  # f - 1 - p
        lca = sb.tile([J, J], f32)
        nc.vector.tensor_scalar(out=lca[:], in0=xc[:], scalar1=negb128[:J, :],
                                scalar2=None, op0=Alu.mult)
        lcb = sb.tile([J, J], f32)
        nc.vector.tensor_scalar(out=lcb[:], in0=xc[:], scalar1=1000.0,
                                scalar2=None, op0=Alu.mult)
        lcm = sb.tile([J, J], f32)
        nc.vector.tensor_tensor(out=lcm[:], in0=lca[:], in1=lcb[:], op=Alu.min)
        lct = sb.tile([J, J], f32)
        nc.scalar.activation(lct[:], lcm[:], Act.Exp)

        # u2: row0 = exp(-b i), row1 = ones (mu term).
        negb01 = sb.tile([2, 1], f32)
        nc.vector.memset(negb01[:], 0.0)
        nc.vector.tensor_copy(negb01[0:1, :], negb[0:1, :])
        xu = sb.tile([2, P], i32)
        nc.gpsimd.iota(xu[:], [[1, P]], base=0, channel_multiplier=0)    # f
        um = sb.tile([2, P], f32)
        nc.vector.tensor_scalar(out=um[:], in0=xu[:], scalar1=negb01[:],
                                scalar2=None, op0=Alu.mult)
        u2 = sb.tile([2, P], f32)
        nc.scalar.activation(u2[:], um[:], Act.Exp)
        u2b = sb.tile([2, P], bf16)
        nc.vector.tensor_copy(u2b[:], u2[:])

        # asb[sl] = b * alpha[s, s], broadcast to all 128 partitions via PE
        asb_row = sb.tile([1, SL], f32)
        nc.vector.tensor_scalar(out=asb_row[:], in0=ads_f[:],
                                scalar1=beta_bc[:1, :], scalar2=None, op0=Alu.mult)
        ones1 = sb.tile([1, P], f32)
        nc.vector.memset(ones1[:], 1.0)
        asb_ps = ps1.tile([P, SL], f32)
        nc.tensor.matmul(asb_ps[:], lhsT=ones1[:], rhs=asb_row[:], start=True, stop=True)
        asb_bc = sb.tile([P, SL], f32)
        nc.vector.tensor_copy(asb_bc[:], asb_ps[:])

        # obs_f[tp, j, sl] = obs * asb[sl]   (convert + scale, 2 halves)
        obs_f = sb.tile([P, XW], bf16)
        obs_f3 = obs_f[:].rearrange("p (j s) -> p j s", s=SL)
        obs_i3 = obs_i[:].rearrange("p (j s) -> p j s", s=SL)
        HB = J // 2
        for h in range(2):
            nc.vector.tensor_tensor(
                out=obs_f3[:, h * HB:(h + 1) * HB, :],
                in0=obs_i3[:, h * HB:(h + 1) * HB, :],
                in1=asb_bc[:].unsqueeze(1).broadcast_to((P, HB, SL)),
                op=Alu.mult,
            )

        # ---- carry chain -------------------------------------------------
        # r[j, sl] = sum_tp vv[tp] * obs_f[tp, j, sl]   (end-of-block sums)
        r_flat = sb.tile([1, XW], f32)
        for h in range(2):
            r_ps = psr.tile([1, 512], f32)
            nc.tensor.matmul(r_ps[:], lhsT=vvb[:],
                             rhs=obs_f[:, h * 512:(h + 1) * 512],
                             start=True, stop=True)
            nc.scalar.activation(r_flat[:, h * 512:(h + 1) * 512], r_ps[:],
                                 Act.Copy)
        r32 = sb.tile([J, SL], f32)
        nc.sync.dma_start(r32[:], r_flat[:])

        c_ps = ps1.tile([J, SL], f32, tag="cps")
        nc.tensor.matmul(c_ps[:], lhsT=lct[:], rhs=r32[:], start=True, stop=True)
        c32 = sb2.tile([J, SL], bf16, tag="c32")
        nc.vector.tensor_copy(c32[:], c_ps[:])

        rhs2 = sb.tile([2, XW], bf16)  # row0 = carry C flat, row1 = mu tiled
        nc.sync.dma_start(rhs2[0:1, :], c32[:])
        mu_b = sb.tile([1, SL], bf16)
        nc.vector.tensor_copy(mu_b[:], mu_f[:])
        nc.scalar.dma_start(
            rhs2[1:2, :].rearrange("o (j s) -> o j s", s=SL),
            mu_b[:].unsqueeze(1).broadcast_to((1, J, SL)))

        # ---- G build (SBUF resident) ------------------------------------
        g_sb = sb.tile([P, XW], f32)
        for h in range(2):
            pch = ps.tile([P, 512], f32)
            nc.tensor.matmul(pch[:], lhsT=ldtb[:],
                             rhs=obs_f[:, h * 512:(h + 1) * 512],
                             start=True, stop=True)
            nc.tensor.matmul(pch[:], lhsT=u2b[:],
                             rhs=rhs2[:, h * 512:(h + 1) * 512],
                             start=False, stop=True, skip_group_check=True)
            if h == 0:
                nc.vector.tensor_copy(g_sb[:, 0:512], pch[:])
            else:
                nc.scalar.activation(g_sb[:, 512:1024], pch[:], Act.Copy)

        # ---- gather + group compress ------------------------------------
        gth = sb.tile([P, NI], f32)
        nc.gpsimd.ap_gather(
            out_ap=gth[:].rearrange("p (i d) -> p i d", d=1),
            in_ap=g_sb[:].rearrange("p (x d) -> p x d", d=1),
            idxs_ap=idxt[:],
            channels=P, num_elems=XW, d=1, num_idxs=NI)

        msked = sb.tile([P, NI], bf16)
        nc.vector.tensor_tensor(out=msked[:], in0=gth[:], in1=mskt[:],
                                op=Alu.mult)

        lam = sb2.tile([NG, NI], f32, tag="lam")
        CC = (512, 512, NI - 1024)
        off = 0
        for ci, cw in enumerate(CC):
            cps = psc.tile([NG, cw], f32)
            nc.tensor.matmul(cps[:], lhsT=grpt[:],
                             rhs=msked[:, off:off + cw], start=True, stop=True)
            nc.vector.tensor_scalar(out=lam[:, off:off + cw], in0=cps[:],
                                    scalar1=float(LAM_MIN), scalar2=None,
                                    op0=Alu.max)
            off += cw
        nc.scalar.dma_start(bass.AP(out_h, 0, [[NI, NG], [1, NI]]), lam[:])

    nc.compile()
    return nc


_NC_CACHE = None


def _get_nc():
    global _NC_CACHE
    if _NC_CACHE is None:
        _NC_CACHE = build_nc()
    return _NC_CACHE


def prepare_in_maps(t, s, obs, mu, alpha, beta):
    """Route queries to cores by s-range; build per-core device inputs.

    Returns (in_maps, perms); perms[c] = (dev_pos, orig_pos) with
    out[orig_pos] = dev_out[dev_pos].
    """
    t = np.ascontiguousarray(np.asarray(t, dtype=np.int32))
    s = np.ascontiguousarray(np.asarray(s, dtype=np.int32))
    obs = np.asarray(obs)
    mu = np.asarray(mu, dtype=np.float32)
    alpha = np.asarray(alpha, dtype=np.float32)
    beta = np.ascontiguousarray(np.asarray(beta, dtype=np.float32))
    adiag = np.ascontiguousarray(np.diagonal(alpha)).astype(np.float32)
    obs8 = obs.astype(np.int8)  # values in [0, 10)

    grp = np.zeros((P, NG), np.float32)
    grp[np.arange(P), np.arange(P) >> 4] = 1.0
    grp = grp.astype(ml_dtypes.bfloat16)

    in_maps, perms = [], []
    for c in range(N_CORES):
        sel = np.nonzero((s >> 5) == c)[0]
        tc_, sc_ = t[sel], s[sel]
        g = (tc_ & 127) >> 4
        x = ((tc_ >> 7) * SL + (sc_ & (SL - 1))).astype(np.int16)
        tl = tc_ & 15
        idxs_dev = np.zeros((P, NI // 16), np.int16)
        mask = np.zeros((P, NI), np.float32)
        dev_pos = np.empty(len(sel), np.int64)
        for gg in range(NG):
            qs = np.nonzero(g == gg)[0]
            n = len(qs)
            if n > NI:
                raise RuntimeError(f"core {c} group {gg}: {n} queries > {NI}")
            i = np.arange(n)
            idxs_dev[16 * gg + (i & 15), i >> 4] = x[qs]
            mask[16 * gg + tl[qs], i] = 1.0
            dev_pos[qs] = gg * NI + i
        obsr = np.ascontiguousarray(
            obs8[:, c * SL:(c + 1) * SL]
            .reshape(J, P, SL).transpose(1, 0, 2).reshape(P, XW))
        in_maps.append({
            "obsr": obsr,
            "idxs": idxs_dev,
            "mask": mask.astype(ml_dtypes.bfloat16),
            "grp": grp,
            "mus": np.ascontiguousarray(mu[c * SL:(c + 1) * SL]),
            "ads": np.ascontiguousarray(adiag[c * SL:(c + 1) * SL]),
            "beta": beta,
        })
        perms.append((dev_pos, sel))
    return in_maps, perms


def finalize(results, perms):
    out = np.empty(BATCH, np.float32)
    for c in range(N_CORES):
        dev = results[c]["out"]
        dev_pos, orig_pos = perms[c]
        out[orig_pos] = dev[dev_pos]
    return out


def kernel(t, s, obs, mu, alpha, beta, **_unused):
    nc = _get_nc()
    in_maps, perms = prepare_in_maps(t, s, obs, mu, alpha, beta)
    res = run_bass_kernel_spmd(nc, in_maps, core_ids=list(range(N_CORES)))
    return finalize(res.results, perms)


if __name__ == "__main__":
    # quick self-check against a numpy re-implementation on random data
    rng = np.random.default_rng(0)
    t = rng.integers(0, N_TIME, BATCH).astype(np.int32)
    s = rng.integers(0, N_SPACE, BATCH).astype(np.int32)
    obs = rng.integers(0, 10, (N_TIME, N_SPACE)).astype(np.int32)
    mu = rng.random(N_SPACE, dtype=np.float32)
    alpha = rng.random((N_SPACE, N_SPACE), dtype=np.float32)
    beta = (rng.random(1, dtype=np.float32) + 0.1).astype(np.float32)

    got = kernel(t=t, s=s, obs=obs, mu=mu, alpha=alpha, beta=beta)

    b = float(beta[0])
    e = np.exp(-b)
    F = np.zeros((N_TIME, N_SPACE), np.float64)
    for tt in range(1, N_TIME):
        F[tt] = e * (F[tt - 1] + obs[tt - 1])
    G = np.clip(mu[None, :] + np.diag(alpha)[None, :] * b * F, LAM_MIN, None)
    want = G[t, s].astype(np.float32)
    err = np.abs(got - want) / np.maximum(np.abs(want), 1e-6)
    print("max rel err:", err.max(), "mean:", err.mean())


# revision 16
# speedup vs baseline: 1.9998x; 1.0656x over previous
"""Discrete Hawkes conditional-intensity kernel for 8 Trainium2 NeuronCores.

Math
----
Reference computes, per query i with (t, s) = (t_i, s_i):

    lam_i = clip(mu[s] + alpha[s, s] * b * F[t, s], 1e-5)
    F[t, s] = sum_{tp < t} obs[tp, s] * exp(-b * (t - tp))

F obeys F[t+1] = e * (F[t] + obs[t]), e = exp(-b): an exponentially-decayed
prefix sum over time.  On device each core builds the table
G[t, sl] = mu[sl] + alpha[sl,sl]*b*F[t, sl] for its 32 s-columns with a
blocked formulation (time blocks of 128 on the PE array + a 32-step
cross-block carry), keeping G in SBUF as [128 (t&127), 1024 ((t>>7)*32+sl)].

Sharding: by SPACE.  Core c owns s-columns [32c, 32c+32) (the time scan is
core-local, so no collectives).  Queries are routed host-side to the core
owning their s value.

Gather: gpsimd ap_gather reads G directly from SBUF -- one instruction for
all ~8.2k queries of a core (no DRAM round-trip, no per-element DMA
descriptors).  ap_gather shares one index list per 16-partition group, so a
query's value appears on all 16 partitions of its group; a host-supplied
one-hot mask + an 8-column matmul compress the group dimension down to the
single wanted value per query.
"""

import os
import sys

import numpy as np

_REPO_CANDIDATES = ("/opt/trn_rl_repo", os.path.expanduser("~/.axon_site/_ro/trn_rl_repo"))
for _p in _REPO_CANDIDATES:
    if os.path.isdir(_p) and _p not in sys.path:
        sys.path.append(_p)

import ml_dtypes
import concourse.bass as bass
import concourse.tile as tile
from concourse import bacc, library_config, mybir
from concourse.bass_utils import run_bass_kernel_spmd

# Problem constants (hardcoded per spec).
N_TIME = 4096
N_SPACE = 256
BATCH = 65536
N_CORES = 8
LAM_MIN = 1e-5

P = 128                  # partitions / time-block size
J = N_TIME // P          # 32 time blocks
SL = N_SPACE // N_CORES  # 32 s-columns per core
XW = J * SL              # 1024 free elements of the G table per partition
NG = 8                   # gpsimd 16-partition groups
NI = 1392                # gather slots per group (mean load 1024, wide margin)
NSLOT = NG * NI          # device output length

f32 = mybir.dt.float32
bf16 = mybir.dt.bfloat16
i32 = mybir.dt.int32
i16 = mybir.dt.int16
i8 = mybir.dt.int8
Alu = mybir.AluOpType
Act = mybir.ActivationFunctionType


def build_nc():
    nc = bacc.Bacc("TRN2", target_bir_lowering=False, debug=False)

    obs_h = nc.dram_tensor("obsr", [P, XW], i8, kind="ExternalInput")
    idx_h = nc.dram_tensor("idxs", [P, NI // 16], i16, kind="ExternalInput")
    msk_h = nc.dram_tensor("mask", [P, NI], bf16, kind="ExternalInput")
    grp_h = nc.dram_tensor("grp", [P, NG], bf16, kind="ExternalInput")
    mus_h = nc.dram_tensor("mus", [SL], f32, kind="ExternalInput")
    ads_h = nc.dram_tensor("ads", [SL], f32, kind="ExternalInput")
    beta_h = nc.dram_tensor("beta", [1], f32, kind="ExternalInput")
    # static integer ramps (host constants; avoids gpsimd iota, whose ucode
    # library would force a mid-kernel swap away from ap_gather's library)
    xd_h = nc.dram_tensor("xd", [P, P], i8, kind="ExternalInput")    # f - p
    xv_h = nc.dram_tensor("xv", [P, 1], i8, kind="ExternalInput")    # p - 128
    xc_h = nc.dram_tensor("xc", [J, J], i8, kind="ExternalInput")    # f - 1 - p
    xu_h = nc.dram_tensor("xu", [2, P], i8, kind="ExternalInput")    # f
    out_h = nc.dram_tensor("out", [NSLOT], f32, kind="ExternalOutput")

    from contextlib import ExitStack

    with tile.TileContext(nc) as tc, ExitStack() as ctx:
        sb = ctx.enter_context(tc.tile_pool(name="sb", bufs=1))
        ps = ctx.enter_context(tc.tile_pool(name="ps", bufs=2, space="PSUM"))
        psr = ctx.enter_context(tc.tile_pool(name="psr", bufs=2, space="PSUM"))
        ps1 = ctx.enter_context(tc.tile_pool(name="ps1", bufs=1, space="PSUM"))
        psc = ctx.enter_context(tc.tile_pool(name="psc", bufs=2, space="PSUM"))
        sb2 = ctx.enter_context(tc.tile_pool(name="sb2", bufs=4))

        # ---- input loads -------------------------------------------------
        # Pin the ap_gather ucode library while Pool DMA state is clean; a
        # mid-kernel swap costs a ~40us drain.
        nc.gpsimd.load_library(library_config.ap_gather)

        beta_bc = sb.tile([P, 1], f32)
        nc.scalar.dma_start(beta_bc[:], bass.AP(beta_h, 0, [[0, P], [1, 1]]))
        mu_f = sb.tile([1, SL], f32)
        nc.scalar.dma_start(mu_f[:], bass.AP(mus_h, 0, [[0, 1], [1, SL]]))
        ads_f = sb.tile([1, SL], f32)
        nc.scalar.dma_start(ads_f[:], bass.AP(ads_h, 0, [[0, 1], [1, SL]]))
        mskt = sb.tile([P, NI], bf16)
        nc.scalar.dma_start(mskt[:], msk_h.ap())
        grpt = sb.tile([P, NG], bf16)
        nc.scalar.dma_start(grpt[:], grp_h.ap())

        obs_i = sb.tile([P, XW], i8)   # [p, (j, sl)] = obs[j*128+p, 32c+sl]
        nc.sync.dma_start(obs_i[:], obs_h.ap())
        xd = sb.tile([P, P], i8)
        nc.sync.dma_start(xd[:], xd_h.ap())
        xc = sb.tile([J, J], i8)
        nc.sync.dma_start(xc[:], xc_h.ap())
        xu = sb.tile([2, P], i8)
        nc.sync.dma_start(xu[:], xu_h.ap())
        xv = sb.tile([P, 1], i8)
        nc.sync.dma_start(xv[:], xv_h.ap())
        idxt = sb.tile([P, NI // 16], i16)
        nc.sync.dma_start(idxt[:], idx_h.ap())

        # ---- runtime constants from beta --------------------------------
        negb = sb.tile([P, 1], f32)
        nc.vector.tensor_scalar(out=negb[:], in0=beta_bc[:], scalar1=-1.0,
                                scalar2=None, op0=Alu.mult)
        negb128 = sb.tile([P, 1], f32)
        nc.vector.tensor_scalar(out=negb128[:], in0=negb[:], scalar1=128.0,
                                scalar2=None, op0=Alu.mult)

        # LdT[tp, m] = exp(-b (m - tp)) for tp < m else 0   (within-block decay)
        lda = sb.tile([P, P], f32)
        nc.vector.tensor_scalar(out=lda[:], in0=xd[:], scalar1=negb[:],
                                scalar2=None, op0=Alu.mult)
        ldb = sb.tile([P, P], f32)
        nc.vector.tensor_scalar(out=ldb[:], in0=xd[:], scalar1=1000.0,
                                scalar2=-1000.0, op0=Alu.mult, op1=Alu.add)
        ldm = sb.tile([P, P], f32)
        nc.vector.tensor_tensor(out=ldm[:], in0=lda[:], in1=ldb[:], op=Alu.min)
        ldt = sb.tile([P, P], f32)
        nc.scalar.activation(ldt[:], ldm[:], Act.Exp)
        ldtb = sb.tile([P, P], bf16)
        nc.vector.tensor_copy(ldtb[:], ldt[:])

        # v[tp] = exp(-b (128 - tp))  (end-of-block carry weights)
        # xv holds p - 128, so multiply by +b to get -b(128 - p).
        vm = sb.tile([P, 1], f32)
        nc.vector.tensor_scalar(out=vm[:], in0=xv[:], scalar1=beta_bc[:],
                                scalar2=None, op0=Alu.mult)
        vv = sb.tile([P, 1], f32)
        nc.scalar.activation(vv[:], vm[:], Act.Exp)
        vvb = sb.tile([P, 1], bf16)
        nc.vector.tensor_copy(vvb[:], vv[:])

        # LcT[k, j] = exp(-128 b (j - 1 - k)) for k <= j-1 else 0  (carry matrix)
        lca = sb.tile([J, J], f32)
        nc.vector.tensor_scalar(out=lca[:], in0=xc[:], scalar1=negb128[:J, :],
                                scalar2=None, op0=Alu.mult)
        lcb = sb.tile([J, J], f32)
        nc.vector.tensor_scalar(out=lcb[:], in0=xc[:], scalar1=1000.0,
                                scalar2=None, op0=Alu.mult)
        lcm = sb.tile([J, J], f32)
        nc.vector.tensor_tensor(out=lcm[:], in0=lca[:], in1=lcb[:], op=Alu.min)
        lct = sb.tile([J, J], f32)
        nc.scalar.activation(lct[:], lcm[:], Act.Exp)

        # u2: row0 = exp(-b i), row1 = ones (mu term).
        negb01 = sb.tile([2, 1], f32)
        nc.vector.memset(negb01[:], 0.0)
        nc.vector.tensor_copy(negb01[0:1, :], negb[0:1, :])
        um = sb.tile([2, P], f32)
        nc.vector.tensor_scalar(out=um[:], in0=xu[:], scalar1=negb01[:],
                                scalar2=None, op0=Alu.mult)
        u2 = sb.tile([2, P], f32)
        nc.scalar.activation(u2[:], um[:], Act.Exp)
        u2b = sb.tile([2, P], bf16)
        nc.vector.tensor_copy(u2b[:], u2[:])

        # asb[sl] = b * alpha[s, s], broadcast to all 128 partitions via PE
        asb_row = sb.tile([1, SL], f32)
        nc.vector.tensor_scalar(out=asb_row[:], in0=ads_f[:],
                                scalar1=beta_bc[:1, :], scalar2=None, op0=Alu.mult)
        ones1 = sb.tile([1, P], f32)
        nc.vector.memset(ones1[:], 1.0)
        asb_ps = ps1.tile([P, SL], f32)
        nc.tensor.matmul(asb_ps[:], lhsT=ones1[:], rhs=asb_row[:], start=True, stop=True)
        asb_bc = sb.tile([P, SL], f32)
        nc.vector.tensor_copy(asb_bc[:], asb_ps[:])

        # obs_f[tp, j, sl] = obs * asb[sl]   (convert + scale, 2 halves)
        obs_f = sb.tile([P, XW], bf16)
        obs_f3 = obs_f[:].rearrange("p (j s) -> p j s", s=SL)
        obs_i3 = obs_i[:].rearrange("p (j s) -> p j s", s=SL)
        HB = J // 2
        for h in range(2):
            nc.vector.tensor_tensor(
                out=obs_f3[:, h * HB:(h + 1) * HB, :],
                in0=obs_i3[:, h * HB:(h + 1) * HB, :],
                in1=asb_bc[:].unsqueeze(1).broadcast_to((P, HB, SL)),
                op=Alu.mult,
            )

        # ---- carry chain -------------------------------------------------
        # r[j, sl] = sum_tp vv[tp] * obs_f[tp, j, sl]   (end-of-block sums)
        r_flat = sb.tile([1, XW], f32)
        for h in range(2):
            r_ps = psr.tile([1, 512], f32)
            nc.tensor.matmul(r_ps[:], lhsT=vvb[:],
                             rhs=obs_f[:, h * 512:(h + 1) * 512],
                             start=True, stop=True)
            if h == 0:
                nc.vector.tensor_copy(r_flat[:, 0:512], r_ps[:])
            else:
                nc.scalar.activation(r_flat[:, 512:1024], r_ps[:], Act.Copy)
        r32 = sb.tile([J, SL], f32)
        nc.sync.dma_start(r32[:], r_flat[:])

        c_ps = ps1.tile([J, SL], f32, tag="cps")
        nc.tensor.matmul(c_ps[:], lhsT=lct[:], rhs=r32[:], start=True, stop=True)
        c32 = sb2.tile([J, SL], bf16, tag="c32")
        nc.vector.tensor_copy(c32[:], c_ps[:])

        rhs2 = sb.tile([2, XW], bf16)  # row0 = carry C flat, row1 = mu tiled
        nc.sync.dma_start(rhs2[0:1, :], c32[:])
        mu_b = sb.tile([1, SL], bf16)
        nc.vector.tensor_copy(mu_b[:], mu_f[:])
        nc.scalar.dma_start(
            rhs2[1:2, :].rearrange("o (j s) -> o j s", s=SL),
            mu_b[:].unsqueeze(1).broadcast_to((1, J, SL)))

        # ---- G build (SBUF resident) ------------------------------------
        g_sb = sb.tile([P, XW], f32)
        for h in range(2):
            pch = ps.tile([P, 512], f32)
            nc.tensor.matmul(pch[:], lhsT=ldtb[:],
                             rhs=obs_f[:, h * 512:(h + 1) * 512],
                             start=True, stop=True)
            nc.tensor.matmul(pch[:], lhsT=u2b[:],
                             rhs=rhs2[:, h * 512:(h + 1) * 512],
                             start=False, stop=True, skip_group_check=True)
            if h == 0:
                nc.vector.tensor_copy(g_sb[:, 0:512], pch[:])
            else:
                nc.scalar.activation(g_sb[:, 512:1024], pch[:], Act.Copy)

        # ---- gather + group compress ------------------------------------
        gth = sb.tile([P, NI], f32)
        nc.gpsimd.ap_gather(
            out_ap=gth[:].rearrange("p (i d) -> p i d", d=1),
            in_ap=g_sb[:].rearrange("p (x d) -> p x d", d=1),
            idxs_ap=idxt[:],
            channels=P, num_elems=XW, d=1, num_idxs=NI)

        msked = sb.tile([P, NI], bf16)
        nc.vector.tensor_tensor(out=msked[:], in0=gth[:], in1=mskt[:],
                                op=Alu.mult)

        lam = sb2.tile([NG, NI], f32, tag="lam")
        CC = (512, 512, NI - 1024)
        off = 0
        for ci, cw in enumerate(CC):
            cps = psc.tile([NG, cw], f32)
            nc.tensor.matmul(cps[:], lhsT=grpt[:],
                             rhs=msked[:, off:off + cw], start=True, stop=True)
            nc.vector.tensor_scalar(out=lam[:, off:off + cw], in0=cps[:],
                                    scalar1=float(LAM_MIN), scalar2=None,
                                    op0=Alu.max)
            off += cw
        nc.scalar.dma_start(bass.AP(out_h, 0, [[NI, NG], [1, NI]]), lam[:])

    nc.compile()
    return nc


_NC_CACHE = None


def _get_nc():
    global _NC_CACHE
    if _NC_CACHE is None:
        _NC_CACHE = build_nc()
    return _NC_CACHE


def prepare_in_maps(t, s, obs, mu, alpha, beta):
    """Route queries to cores by s-range; build per-core device inputs.

    Returns (in_maps, perms); perms[c] = (dev_pos, orig_pos) with
    out[orig_pos] = dev_out[dev_pos].
    """
    t = np.ascontiguousarray(np.asarray(t, dtype=np.int32))
    s = np.ascontiguousarray(np.asarray(s, dtype=np.int32))
    obs = np.asarray(obs)
    mu = np.asarray(mu, dtype=np.float32)
    alpha = np.asarray(alpha, dtype=np.float32)
    beta = np.ascontiguousarray(np.asarray(beta, dtype=np.float32))
    adiag = np.ascontiguousarray(np.diagonal(alpha)).astype(np.float32)
    obs8 = obs.astype(np.int8)  # values in [0, 10)

    grp = np.zeros((P, NG), np.float32)
    grp[np.arange(P), np.arange(P) >> 4] = 1.0
    grp = grp.astype(ml_dtypes.bfloat16)

    pp = np.arange(P, dtype=np.int32)
    ff = np.arange(P, dtype=np.int32)
    xd = (ff[None, :] - pp[:, None]).astype(np.int8)                 # f - p
    xv = (pp[:, None] - P).astype(np.int8)                           # p - 128
    kk = np.arange(J, dtype=np.int32)
    xc = (kk[None, :] - 1 - kk[:, None]).astype(np.int8)             # f - 1 - p
    xu = np.broadcast_to(ff[None, :], (2, P)).astype(np.int8).copy() # f

    in_maps, perms = [], []
    for c in range(N_CORES):
        sel = np.nonzero((s >> 5) == c)[0]
        tc_, sc_ = t[sel], s[sel]
        g = (tc_ & 127) >> 4
        x = ((tc_ >> 7) * SL + (sc_ & (SL - 1))).astype(np.int16)
        tl = tc_ & 15
        idxs_dev = np.zeros((P, NI // 16), np.int16)
        mask = np.zeros((P, NI), np.float32)
        dev_pos = np.empty(len(sel), np.int64)
        for gg in range(NG):
            qs = np.nonzero(g == gg)[0]
            n = len(qs)
            if n > NI:
                raise RuntimeError(f"core {c} group {gg}: {n} queries > {NI}")
            i = np.arange(n)
            idxs_dev[16 * gg + (i & 15), i >> 4] = x[qs]
            mask[16 * gg + tl[qs], i] = 1.0
            dev_pos[qs] = gg * NI + i
        obsr = np.ascontiguousarray(
            obs8[:, c * SL:(c + 1) * SL]
            .reshape(J, P, SL).transpose(1, 0, 2).reshape(P, XW))
        in_maps.append({
            "obsr": obsr,
            "idxs": idxs_dev,
            "mask": mask.astype(ml_dtypes.bfloat16),
            "grp": grp,
            "mus": np.ascontiguousarray(mu[c * SL:(c + 1) * SL]),
            "ads": np.ascontiguousarray(adiag[c * SL:(c + 1) * SL]),
            "beta": beta,
            "xd": xd, "xv": xv, "xc": xc, "xu": xu,
        })
        perms.append((dev_pos, sel))
    return in_maps, perms


def finalize(results, perms):
    out = np.empty(BATCH, np.float32)
    for c in range(N_CORES):
        dev = results[c]["out"]
        dev_pos, orig_pos = perms[c]
        out[orig_pos] = dev[dev_pos]
    return out


def kernel(t, s, obs, mu, alpha, beta, **_unused):
    nc = _get_nc()
    in_maps, perms = prepare_in_maps(t, s, obs, mu, alpha, beta)
    res = run_bass_kernel_spmd(nc, in_maps, core_ids=list(range(N_CORES)))
    return finalize(res.results, perms)


if __name__ == "__main__":
    # quick self-check against a numpy re-implementation on random data
    rng = np.random.default_rng(0)
    t = rng.integers(0, N_TIME, BATCH).astype(np.int32)
    s = rng.integers(0, N_SPACE, BATCH).astype(np.int32)
    obs = rng.integers(0, 10, (N_TIME, N_SPACE)).astype(np.int32)
    mu = rng.random(N_SPACE, dtype=np.float32)
    alpha = rng.random((N_SPACE, N_SPACE), dtype=np.float32)
    beta = (rng.random(1, dtype=np.float32) + 0.1).astype(np.float32)

    got = kernel(t=t, s=s, obs=obs, mu=mu, alpha=alpha, beta=beta)

    b = float(beta[0])
    e = np.exp(-b)
    F = np.zeros((N_TIME, N_SPACE), np.float64)
    for tt in range(1, N_TIME):
        F[tt] = e * (F[tt - 1] + obs[tt - 1])
    G = np.clip(mu[None, :] + np.diag(alpha)[None, :] * b * F, LAM_MIN, None)
    want = G[t, s].astype(np.float32)
    err = np.abs(got - want) / np.maximum(np.abs(want), 1e-6)
    print("max rel err:", err.max(), "mean:", err.mean())
